# revision 22
# baseline (speedup 1.0000x reference)
"""Trainium2 Bass kernel for quantized int8 per-channel Conv2d.

Reference semantics (fp32):
  x_f = (x_int8 - 7) * 0.01                      # per-tensor dequant
  w_f = (w_int8 - zp[cout]) * scale[cout]        # per-channel dequant
  y   = round(conv2d_valid(x_f, w_f) + bias[cout])  -> int32

Winograd F(2,3) along W (direct conv along H), exact in fp16:
  Per 2 output cols j..j+1 and tap row dh, with g = w - zp (host-side):
    V0 = x0-x2, V1 = x1+x2, V2 = x2-x1, V3 = x1-x3   (ints <= 270: fp16-exact)
    U0 = g0, U1 = (g0+g1+g2)/2, U2 = (g0-g1+g2)/2, U3 = g2  (halves: fp16-exact)
    y0 = sum m0+m1+m2,  y1 = sum m1-m2-m3,  m_u = V_u . U_u  (over cin, dh)
  12 column-streams per output pixel per m-tile instead of 18 -> 0.667x PE time.
  The x-7 offset cancels in V0/V2/V3 and contributes a per-cout constant via
  V1 (since 2*sum U1 = sum g exactly), folded into bias on the host:
    bias_f = bias - 0.07*scale*sum(w - zp).
  Products are half-integers < 2^17, accumulated exactly in fp32 PSUM; final
  affine + magic-number round matches jnp.round to ~1e-5 rel (half-ULP ties).

Sharding: data-parallel over batch N=32 across 8 cores (4 images each);
weights/scales/bias replicated.
"""

import numpy as np

import concourse.bass as bass
import concourse.mybir as mybir
from concourse import bacc
from concourse.tile import TileContext
from concourse.bass_utils import run_bass_kernel_spmd

# Problem shapes (hardcoded per contract)
N, CIN, H, W = 32, 256, 56, 56
COUT, KH, KW = 256, 3, 3
HO, WO = H - KH + 1, W - KW + 1          # 54, 54
NCORES = 8
NPER = N // NCORES                        # images per core
HW = H * W                                # 3136
KT = CIN // 128                           # 2 cin tiles
MT = COUT // 128                          # 2 cout tiles
NU = 4                                    # winograd transform size
JW = WO // 2                              # 27 output col-tiles
NSET = KH * NU                            # 12 weight sets per cin tile
ROWS_C = 18                               # output rows per chunk
NCHUNK = HO // ROWS_C                     # 3
CHUNK = ROWS_C * JW                       # 486 psum cols per u-component
MAGIC = 12582912.0                        # 1.5 * 2**23  (fp32 RNE rounding trick)

_CACHE = {}


def _build_program():
    nc = bacc.Bacc("TRN2", target_bir_lowering=False, debug=False,
                   num_devices=NCORES)
    dt = mybir.dt

    x_d = nc.dram_tensor("x", [NPER, CIN, H, W], dt.int8, kind="ExternalInput")
    wt_d = nc.dram_tensor("wt", [NSET, CIN, COUT], dt.float16,
                          kind="ExternalInput")
    sc_d = nc.dram_tensor("scales", [COUT], dt.float32, kind="ExternalInput")
    bi_d = nc.dram_tensor("bias", [COUT], dt.float32, kind="ExternalInput")
    out_d = nc.dram_tensor("out", [NPER, COUT, HO, WO], dt.int32,
                           kind="ExternalOutput")

    with TileContext(nc) as tc:
        with (
            tc.tile_pool(name="const", bufs=1) as cpool,
            tc.tile_pool(name="xin", bufs=2) as xpool,
            tc.tile_pool(name="vwin", bufs=3) as vpool,
            tc.tile_pool(name="psum", bufs=8, space="PSUM") as ppool,
            tc.tile_pool(name="tmp", bufs=8) as tpool,
            tc.tile_pool(name="outb", bufs=6) as opool,
        ):
            # PE warm-up: tiny matmuls fill the initial DMA wait, flipping
            # the HAM clock gate to 8/8 before the first real matmul.
            wupw = cpool.tile([128, 1], dt.bfloat16)
            nc.vector.memset(wupw[:, :], 1.0)
            wupx = cpool.tile([128, CHUNK], dt.bfloat16)
            nc.vector.memset(wupx[:, :], 1.0)
            # warm-up psum shares the main "ps" ring (8 banks total; each
            # accumulation series must own a whole bank: a start=True matmul
            # resets pending-zero state at 2KB bank granularity).
            # Full-width (N=486) warm-up matmuls keep the PE busy enough to
            # flip the HAM clock gate to 8/8 before the first real matmul.
            wups = ppool.tile([128, CHUNK], dt.float32, name="ps", tag="ps")
            for _ in range(8):
                nc.tensor.matmul(wups[0:1, :], wupw[:, :], wupx[:, :],
                                 start=True, stop=True)

            # pre-transformed fp16 weights: [set, cin, cout] -> lhsT tiles
            wb = cpool.tile([128, KT, NSET, COUT], dt.float16)

            # combined output scale 0.01*scale[o] and folded bias (DMA'd
            # inside load_image(0) after the critical first x/w pieces)
            sc2 = cpool.tile([128, MT], dt.float32)
            bi2 = cpool.tile([128, MT], dt.float32)

            def xtile():
                return xpool.tile([128, KT, HW], dt.int8, name="xi")

            def vtile():
                return vpool.tile([128, KT, NU, H, JW], dt.float16, name="vb")

            def xdma(xi, n, k, r0, r1):
                nc.sync.dma_start(
                    out=xi[:, k, r0 * W:r1 * W],
                    in_=x_d[n, k * 128:(k + 1) * 128].rearrange(
                        "p h w -> p (h w)")[:, r0 * W:r1 * W])

            def vcalc(xi, vb, k, r0, r1, split=False):
                # F(2,3) input transform on row range [r0, r1).  For
                # prefetched images (a full phase of slack) V2/V3 run on the
                # otherwise-idle GpSimd engine to relieve the DVE.
                xv = xi[:, k, :].rearrange("p (h j t) -> p h j t", t=2, j=28)
                x0 = xv[:, r0:r1, 0:JW, 0]
                x1 = xv[:, r0:r1, 0:JW, 1]
                x2 = xv[:, r0:r1, 1:JW + 1, 0]
                x3 = xv[:, r0:r1, 1:JW + 1, 1]
                sub = mybir.AluOpType.subtract
                add = mybir.AluOpType.add
                eng = nc.vector
                nc.vector.tensor_tensor(vb[:, k, 0, r0:r1, :], x0, x2, sub)
                nc.vector.tensor_tensor(vb[:, k, 1, r0:r1, :], x1, x2, add)
                eng.tensor_tensor(vb[:, k, 2, r0:r1, :], x2, x1, sub)
                eng.tensor_tensor(vb[:, k, 3, r0:r1, :], x1, x3, sub)

            def load_image(n, head=False):
                xi = xtile()
                vb = vtile()
                if head:
                    # critical path to the first matmul: x(k0) rows then the
                    # k0 weights; everything else is interleaved behind them
                    xdma(xi, n, 0, 0, 28)
                    nc.sync.dma_start(
                        out=wb[:, 0, :, 0:128],
                        in_=wt_d[:, 0:128, 0:128].rearrange("t p o -> p t o"))
                    vcalc(xi, vb, 0, 0, 28)
                    nc.sync.dma_start(
                        out=wb[:, 0, :, 128:256],
                        in_=wt_d[:, 0:128, 128:256].rearrange("t p o -> p t o"))
                    xdma(xi, n, 0, 28, 56)
                    vcalc(xi, vb, 0, 28, 56)
                    xdma(xi, n, 1, 0, 28)
                    vcalc(xi, vb, 1, 0, 28)
                    nc.sync.dma_start(
                        out=wb[:, 1, :, :],
                        in_=wt_d[:, 128:256, :].rearrange("t p o -> p t o"))
                    xdma(xi, n, 1, 28, 56)
                    vcalc(xi, vb, 1, 28, 56)
                    nc.sync.dma_start(out=sc2[:, :],
                                      in_=sc_d.rearrange("(m p) -> p m", p=128))
                    nc.sync.dma_start(out=bi2[:, :],
                                      in_=bi_d.rearrange("(m p) -> p m", p=128))
                else:
                    for k in range(KT):
                        xdma(xi, n, k, 0, 56)
                        vcalc(xi, vb, k, 0, 56)
                return vb

            def load_x(n):
                xi = xtile()
                for k in range(KT):
                    xdma(xi, n, k, 0, 56)
                return xi

            def calc_v(xi, vb, k):
                vcalc(xi, vb, k, 0, 56)

            def emit_chunk(vb, n, m, c, r0=0, nr=ROWS_C):
                # one psum bank per u-series: interleaved start=True matmuls
                # in a shared bank would wipe each other's accumulation.
                # Tiles are always full-bank; sub-chunks (nr < ROWS_C, used to
                # shorten the kernel tail) just use the first nr*JW columns.
                nj = nr * JW
                base = ROWS_C * c + r0
                ps = [ppool.tile([128, CHUNK], dt.float32, name="ps",
                                 tag="ps") for _ in range(NU)]
                for k in range(KT):
                    for dh in range(KH):
                        for u in range(NU):
                            rhs = vb[:, k, u, base + dh:base + dh + nr, :]
                            nc.tensor.matmul(
                                ps[u][:, 0:nj],
                                wb[:, k, dh * NU + u, m * 128:(m + 1) * 128],
                                rhs, start=(k == 0 and dh == 0),
                                stop=(k == KT - 1 and dh == KH - 1))

                sub = mybir.AluOpType.subtract
                add = mybir.AluOpType.add
                m0, m1, m2, m3 = (ps[u][:, 0:nj] for u in range(NU))
                # ACT stages m1 and m0 to SBUF: DVE has one PSUM read port
                # (two-PSUM tensor_tensor is illegal), and early copies free
                # the psum banks sooner for the ring
                a1 = tpool.tile([128, CHUNK], dt.float32)
                nc.scalar.copy(a1[:, 0:nj], m1)
                t1 = tpool.tile([128, CHUNK], dt.float32)
                t2 = tpool.tile([128, CHUNK], dt.float32)
                nc.vector.tensor_tensor(t1[:, 0:nj], a1[:, 0:nj], m2, add)
                nc.vector.tensor_tensor(t2[:, 0:nj], a1[:, 0:nj], m2, sub)
                yf = tpool.tile([128, ROWS_C, WO], dt.float32)
                yv = yf[:, 0:nr, :].rearrange("p r (j t) -> p r j t", t=2)
                nc.vector.tensor_tensor(
                    yv[:, :, :, 0], m0.rearrange("p (r j) -> p r j", j=JW),
                    t1[:, 0:nj].rearrange("p (r j) -> p r j", j=JW), add)
                nc.vector.tensor_tensor(
                    yv[:, :, :, 1], t2[:, 0:nj].rearrange("p (r j) -> p r j", j=JW),
                    m3.rearrange("p (r j) -> p r j", j=JW), sub)
                # y = round(0.01*scale*Y + bias_f) -> int32, all in one ACT op
                # (the fp32->int32 output conversion rounds to nearest even)
                ob = opool.tile([128, ROWS_C, WO], dt.int32)
                nc.scalar.activation(
                    ob[:, 0:nr, :].rearrange("p r w -> p (r w)"),
                    yf[:, 0:nr, :].rearrange("p r w -> p (r w)"),
                    mybir.ActivationFunctionType.Identity,
                    bias=bi2[:, m:m + 1], scale=sc2[:, m:m + 1])
                nc.sync.dma_start(
                    out=out_d[n, m * 128:(m + 1) * 128, base:base + nr, :],
                    in_=ob[:, 0:nr, :])

            # ---- per-image pipeline ----
            # Interleave m=1 of image n with m=0 of image n+1 chunk-by-chunk:
            # psum-ring reuse distance stays at 2 chunks everywhere (a plain
            # per-image loop collapses it at image boundaries, stalling the
            # PE ~7us and re-throttling the HAM clock gate). Image n+2's x
            # DMA (c==0) and V transform (c==1) are staged so V ops never
            # head-of-line-block the DVE queue behind a pending DMA.
            vbs = [None] * NPER
            xis = [None] * NPER
            vbs[0] = load_image(0, head=True)
            if NPER > 1:
                # image 1's V transform has no phase of slack: its x DMA and
                # k0 transform fill the head's dead DVE window, k1 spreads
                # over the m0(0) chunks
                xis[1] = load_x(1)
                vbs[1] = vtile()
                vcalc(xis[1], vbs[1], 0, 0, 56)
            for c in range(NCHUNK):
                emit_chunk(vbs[0], 0, 0, c)
                if NPER > 1 and c < 2:
                    vcalc(xis[1], vbs[1], 1, 28 * c, 28 * (c + 1))
            for n in range(NPER):
                lastimg = n == NPER - 1
                for c in range(NCHUNK):
                    if lastimg and c >= NCHUNK - 2:
                        # split the final chunks so each half's epilogue
                        # overlaps the next half's matmuls (shorter tail)
                        emit_chunk(vbs[n], n, 1, c, 0, ROWS_C // 2)
                        emit_chunk(vbs[n], n, 1, c, ROWS_C // 2, ROWS_C // 2)
                    else:
                        emit_chunk(vbs[n], n, 1, c)
                    if not lastimg:
                        emit_chunk(vbs[n + 1], n + 1, 0, c)
                        if n + 2 < NPER:
                            if c == 0:
                                xis[n + 2] = load_x(n + 2)
                                vbs[n + 2] = vtile()
                            else:
                                calc_v(xis[n + 2], vbs[n + 2], c - 1)

    nc.compile()
    return nc


def _prep_weights(w, zp):
    # host-side: g = w - zp (per cout), then G-transform along kw; all
    # values are halves <= 205.5 -> exact in fp16
    g = w.astype(np.float64) - zp.astype(np.float64)[:, None, None, None]
    u = np.empty((KH, NU, CIN, COUT), dtype=np.float64)
    for dh in range(KH):
        gd = g[:, :, dh, :]                       # [cout, cin, kw]
        u[dh, 0] = gd[:, :, 0].T
        u[dh, 1] = ((gd[:, :, 0] + gd[:, :, 1] + gd[:, :, 2]) / 2).T
        u[dh, 2] = ((gd[:, :, 0] - gd[:, :, 1] + gd[:, :, 2]) / 2).T
        u[dh, 3] = gd[:, :, 2].T
    # [set = dh*NU+u, cin, cout]
    return np.ascontiguousarray(
        u.reshape(NSET, CIN, COUT).astype(np.float16))


def _prep_scalars(w, zp, scales, bias):
    g64 = (w.astype(np.float64)
           - zp.astype(np.float64)[:, None, None, None]).sum(axis=(1, 2, 3))
    sc = (0.01 * scales.astype(np.float64)).astype(np.float32)
    bi = (bias.astype(np.float64)
          - 0.07 * scales.astype(np.float64) * g64).astype(np.float32)
    return sc, bi


def kernel(**inputs) -> np.ndarray:
    x = np.ascontiguousarray(np.asarray(inputs["inputVec"], dtype=np.int8))
    w = np.asarray(inputs["weight"], dtype=np.int8)
    scales = np.asarray(inputs["scales"], dtype=np.float32)
    zp = np.asarray(inputs["zeropoints"], dtype=np.int32)
    bias = np.asarray(inputs["bias"], dtype=np.float32)
    assert x.shape == (N, CIN, H, W) and w.shape == (COUT, CIN, KH, KW)

    wt = _prep_weights(w, zp)
    sc, bi = _prep_scalars(w, zp, scales, bias)

    if "nc" not in _CACHE:
        _CACHE["nc"] = _build_program()
    nc = _CACHE["nc"]

    in_maps = [
        {"x": x[c * NPER:(c + 1) * NPER], "wt": wt, "scales": sc, "bias": bi}
        for c in range(NCORES)
    ]
    res = run_bass_kernel_spmd(nc, in_maps, list(range(NCORES)))
    out = np.concatenate([res.results[c]["out"] for c in range(NCORES)], axis=0)
    return out


# revision 23
# speedup vs baseline: 1.0387x; 1.0387x over previous
"""Trainium2 Bass kernel for quantized int8 per-channel Conv2d.

Reference semantics (fp32):
  x_f = (x_int8 - 7) * 0.01                      # per-tensor dequant
  w_f = (w_int8 - zp[cout]) * scale[cout]        # per-channel dequant
  y   = round(conv2d_valid(x_f, w_f) + bias[cout])  -> int32

Winograd F(2,3) along W (direct conv along H), exact in fp16:
  Per 2 output cols j..j+1 and tap row dh, with g = w - zp (host-side):
    V0 = x0-x2, V1 = x1+x2, V2 = x2-x1, V3 = x1-x3   (ints <= 270: fp16-exact)
    U0 = g0, U1 = (g0+g1+g2)/2, U2 = (g0-g1+g2)/2, U3 = g2  (halves: fp16-exact)
    y0 = sum m0+m1+m2,  y1 = sum m1-m2-m3,  m_u = V_u . U_u  (over cin, dh)
  12 column-streams per output pixel per m-tile instead of 18 -> 0.667x PE time.
  The x-7 offset cancels in V0/V2/V3 and contributes a per-cout constant via
  V1 (since 2*sum U1 = sum g exactly), folded into bias on the host:
    bias_f = bias - 0.07*scale*sum(w - zp).
  Products are half-integers < 2^17, accumulated exactly in fp32 PSUM; final
  affine + magic-number round matches jnp.round to ~1e-5 rel (half-ULP ties).

Sharding: data-parallel over batch N=32 across 8 cores (4 images each);
weights/scales/bias replicated.
"""

import numpy as np

import concourse.bass as bass
import concourse.mybir as mybir
from concourse import bacc
from concourse.tile import TileContext
from concourse.bass_utils import run_bass_kernel_spmd

# Problem shapes (hardcoded per contract)
N, CIN, H, W = 32, 256, 56, 56
COUT, KH, KW = 256, 3, 3
HO, WO = H - KH + 1, W - KW + 1          # 54, 54
NCORES = 8
NPER = N // NCORES                        # images per core
HW = H * W                                # 3136
KT = CIN // 128                           # 2 cin tiles
MT = COUT // 128                          # 2 cout tiles
NU = 4                                    # winograd transform size
JW = WO // 2                              # 27 output col-tiles
NSET = KH * NU                            # 12 weight sets per cin tile
ROWS_C = 18                               # output rows per chunk
NCHUNK = HO // ROWS_C                     # 3
CHUNK = ROWS_C * JW                       # 486 psum cols per u-component
MAGIC = 12582912.0                        # 1.5 * 2**23  (fp32 RNE rounding trick)

_CACHE = {}


def _build_program():
    nc = bacc.Bacc("TRN2", target_bir_lowering=False, debug=False,
                   num_devices=NCORES)
    dt = mybir.dt

    x_d = nc.dram_tensor("x", [NPER, CIN, H, W], dt.int8, kind="ExternalInput")
    wt_d = nc.dram_tensor("wt", [NSET, CIN, COUT], dt.float16,
                          kind="ExternalInput")
    sc_d = nc.dram_tensor("scales", [COUT], dt.float32, kind="ExternalInput")
    bi_d = nc.dram_tensor("bias", [COUT], dt.float32, kind="ExternalInput")
    out_d = nc.dram_tensor("out", [NPER, COUT, HO, WO], dt.int32,
                           kind="ExternalOutput")

    with TileContext(nc) as tc:
        with (
            tc.tile_pool(name="const", bufs=1) as cpool,
            tc.tile_pool(name="xin", bufs=2) as xpool,
            tc.tile_pool(name="vwin", bufs=3) as vpool,
            tc.tile_pool(name="psum", bufs=8, space="PSUM") as ppool,
            tc.tile_pool(name="tmp", bufs=8) as tpool,
            tc.tile_pool(name="outb", bufs=6) as opool,
        ):
            # PE warm-up: tiny matmuls fill the initial DMA wait, flipping
            # the HAM clock gate to 8/8 before the first real matmul.
            wupw = cpool.tile([128, 1], dt.bfloat16)
            nc.vector.memset(wupw[:, :], 1.0)
            wupx = cpool.tile([128, CHUNK], dt.bfloat16)
            nc.vector.memset(wupx[:, :], 1.0)
            # warm-up psum shares the main "ps" ring (8 banks total; each
            # accumulation series must own a whole bank: a start=True matmul
            # resets pending-zero state at 2KB bank granularity).
            # Full-width (N=486) warm-up matmuls keep the PE busy enough to
            # flip the HAM clock gate to 8/8 before the first real matmul.
            wups = ppool.tile([128, CHUNK], dt.float32, name="ps", tag="ps")
            for _ in range(8):
                nc.tensor.matmul(wups[0:1, :], wupw[:, :], wupx[:, :],
                                 start=True, stop=True)

            # pre-transformed fp16 weights: [set, cin, cout] -> lhsT tiles
            wb = cpool.tile([128, KT, NSET, COUT], dt.float16)

            # combined output scale 0.01*scale[o] and folded bias (DMA'd
            # inside load_image(0) after the critical first x/w pieces)
            sc2 = cpool.tile([128, MT], dt.float32)
            bi2 = cpool.tile([128, MT], dt.float32)

            def xtile():
                return xpool.tile([128, KT, HW], dt.int8, name="xi")

            def vtile():
                return vpool.tile([128, KT, NU, H, JW], dt.float16, name="vb")

            def xdma(xi, n, k, r0, r1):
                nc.sync.dma_start(
                    out=xi[:, k, r0 * W:r1 * W],
                    in_=x_d[n, k * 128:(k + 1) * 128].rearrange(
                        "p h w -> p (h w)")[:, r0 * W:r1 * W])

            def vcalc(xi, vb, k, r0, r1, split=False):
                # F(2,3) input transform on row range [r0, r1).  For
                # prefetched images (a full phase of slack) V2/V3 run on the
                # otherwise-idle GpSimd engine to relieve the DVE.
                xv = xi[:, k, :].rearrange("p (h j t) -> p h j t", t=2, j=28)
                x0 = xv[:, r0:r1, 0:JW, 0]
                x1 = xv[:, r0:r1, 0:JW, 1]
                x2 = xv[:, r0:r1, 1:JW + 1, 0]
                x3 = xv[:, r0:r1, 1:JW + 1, 1]
                sub = mybir.AluOpType.subtract
                add = mybir.AluOpType.add
                eng = nc.vector
                nc.vector.tensor_tensor(vb[:, k, 0, r0:r1, :], x0, x2, sub)
                nc.vector.tensor_tensor(vb[:, k, 1, r0:r1, :], x1, x2, add)
                eng.tensor_tensor(vb[:, k, 2, r0:r1, :], x2, x1, sub)
                eng.tensor_tensor(vb[:, k, 3, r0:r1, :], x1, x3, sub)

            def load_image(n, head=False):
                xi = xtile()
                vb = vtile()
                if head:
                    # critical path to the first matmul: x(k0) rows then the
                    # k0 weights; V pieces ordered by first consumption
                    # (chunk 0 needs k0/k1 rows 0:20, chunk 1 rows 18:38)
                    xdma(xi, n, 0, 0, 28)
                    nc.sync.dma_start(
                        out=wb[:, 0, :, 0:128],
                        in_=wt_d[:, 0:128, 0:128].rearrange("t p o -> p t o"))
                    vcalc(xi, vb, 0, 0, 28)
                    xdma(xi, n, 1, 0, 28)
                    nc.sync.dma_start(
                        out=wb[:, 1, :, 0:128],
                        in_=wt_d[:, 128:256, 0:128].rearrange("t p o -> p t o"))
                    vcalc(xi, vb, 1, 0, 28)
                    xdma(xi, n, 0, 28, 56)
                    nc.sync.dma_start(
                        out=wb[:, 0, :, 128:256],
                        in_=wt_d[:, 0:128, 128:256].rearrange("t p o -> p t o"))
                    vcalc(xi, vb, 0, 28, 56)
                    xdma(xi, n, 1, 28, 56)
                    nc.sync.dma_start(
                        out=wb[:, 1, :, 128:256],
                        in_=wt_d[:, 128:256, 128:256].rearrange("t p o -> p t o"))
                    vcalc(xi, vb, 1, 28, 56)
                    nc.sync.dma_start(out=sc2[:, :],
                                      in_=sc_d.rearrange("(m p) -> p m", p=128))
                    nc.sync.dma_start(out=bi2[:, :],
                                      in_=bi_d.rearrange("(m p) -> p m", p=128))
                else:
                    for k in range(KT):
                        xdma(xi, n, k, 0, 56)
                        vcalc(xi, vb, k, 0, 56)
                return vb

            def load_x(n):
                xi = xtile()
                for k in range(KT):
                    xdma(xi, n, k, 0, 56)
                return xi

            def calc_v(xi, vb, k):
                vcalc(xi, vb, k, 0, 56)

            def emit_chunk(vb, n, m, c, r0=0, nr=ROWS_C):
                # one psum bank per u-series: interleaved start=True matmuls
                # in a shared bank would wipe each other's accumulation.
                # Tiles are always full-bank; sub-chunks (nr < ROWS_C, used to
                # shorten the kernel tail) just use the first nr*JW columns.
                nj = nr * JW
                base = ROWS_C * c + r0
                ps = [ppool.tile([128, CHUNK], dt.float32, name="ps",
                                 tag="ps") for _ in range(NU)]
                for k in range(KT):
                    for dh in range(KH):
                        for u in range(NU):
                            rhs = vb[:, k, u, base + dh:base + dh + nr, :]
                            nc.tensor.matmul(
                                ps[u][:, 0:nj],
                                wb[:, k, dh * NU + u, m * 128:(m + 1) * 128],
                                rhs, start=(k == 0 and dh == 0),
                                stop=(k == KT - 1 and dh == KH - 1))

                sub = mybir.AluOpType.subtract
                add = mybir.AluOpType.add
                m0, m1, m2, m3 = (ps[u][:, 0:nj] for u in range(NU))
                # ACT stages m1 and m0 to SBUF: DVE has one PSUM read port
                # (two-PSUM tensor_tensor is illegal), and early copies free
                # the psum banks sooner for the ring
                a1 = tpool.tile([128, CHUNK], dt.float32)
                nc.scalar.copy(a1[:, 0:nj], m1)
                t1 = tpool.tile([128, CHUNK], dt.float32)
                t2 = tpool.tile([128, CHUNK], dt.float32)
                nc.vector.tensor_tensor(t1[:, 0:nj], a1[:, 0:nj], m2, add)
                nc.vector.tensor_tensor(t2[:, 0:nj], a1[:, 0:nj], m2, sub)
                yf = tpool.tile([128, ROWS_C, WO], dt.float32)
                yv = yf[:, 0:nr, :].rearrange("p r (j t) -> p r j t", t=2)
                nc.vector.tensor_tensor(
                    yv[:, :, :, 0], m0.rearrange("p (r j) -> p r j", j=JW),
                    t1[:, 0:nj].rearrange("p (r j) -> p r j", j=JW), add)
                nc.vector.tensor_tensor(
                    yv[:, :, :, 1], t2[:, 0:nj].rearrange("p (r j) -> p r j", j=JW),
                    m3.rearrange("p (r j) -> p r j", j=JW), sub)
                # y = round(0.01*scale*Y + bias_f) -> int32, all in one ACT op
                # (the fp32->int32 output conversion rounds to nearest even)
                ob = opool.tile([128, ROWS_C, WO], dt.int32)
                nc.scalar.activation(
                    ob[:, 0:nr, :].rearrange("p r w -> p (r w)"),
                    yf[:, 0:nr, :].rearrange("p r w -> p (r w)"),
                    mybir.ActivationFunctionType.Identity,
                    bias=bi2[:, m:m + 1], scale=sc2[:, m:m + 1])
                nc.sync.dma_start(
                    out=out_d[n, m * 128:(m + 1) * 128, base:base + nr, :],
                    in_=ob[:, 0:nr, :])

            # ---- per-image pipeline ----
            # Interleave m=1 of image n with m=0 of image n+1 chunk-by-chunk:
            # psum-ring reuse distance stays at 2 chunks everywhere (a plain
            # per-image loop collapses it at image boundaries, stalling the
            # PE ~7us and re-throttling the HAM clock gate). Image n+2's x
            # DMA (c==0) and V transform (c==1) are staged so V ops never
            # head-of-line-block the DVE queue behind a pending DMA.
            vbs = [None] * NPER
            xis = [None] * NPER
            vbs[0] = load_image(0, head=True)
            for c in range(NCHUNK):
                emit_chunk(vbs[0], 0, 0, c)
                if NPER > 1:
                    if c == 0:
                        xis[1] = load_x(1)
                        vbs[1] = vtile()
                    else:
                        calc_v(xis[1], vbs[1], c - 1)
            for n in range(NPER):
                lastimg = n == NPER - 1
                for c in range(NCHUNK):
                    if lastimg and c == NCHUNK - 1:
                        # split the final chunks so each half's epilogue
                        # overlaps the next half's matmuls (shorter tail)
                        emit_chunk(vbs[n], n, 1, c, 0, ROWS_C // 2)
                        emit_chunk(vbs[n], n, 1, c, ROWS_C // 2, ROWS_C // 2)
                    else:
                        emit_chunk(vbs[n], n, 1, c)
                    if not lastimg:
                        emit_chunk(vbs[n + 1], n + 1, 0, c)
                        if n + 2 < NPER:
                            if c == 0:
                                xis[n + 2] = load_x(n + 2)
                                vbs[n + 2] = vtile()
                            else:
                                calc_v(xis[n + 2], vbs[n + 2], c - 1)

    nc.compile()
    return nc


def _prep_weights(w, zp):
    # host-side: g = w - zp (per cout), then G-transform along kw; all
    # values are halves <= 205.5 -> exact in fp16
    g = w.astype(np.float64) - zp.astype(np.float64)[:, None, None, None]
    u = np.empty((KH, NU, CIN, COUT), dtype=np.float64)
    for dh in range(KH):
        gd = g[:, :, dh, :]                       # [cout, cin, kw]
        u[dh, 0] = gd[:, :, 0].T
        u[dh, 1] = ((gd[:, :, 0] + gd[:, :, 1] + gd[:, :, 2]) / 2).T
        u[dh, 2] = ((gd[:, :, 0] - gd[:, :, 1] + gd[:, :, 2]) / 2).T
        u[dh, 3] = gd[:, :, 2].T
    # [set = dh*NU+u, cin, cout]
    return np.ascontiguousarray(
        u.reshape(NSET, CIN, COUT).astype(np.float16))


def _prep_scalars(w, zp, scales, bias):
    g64 = (w.astype(np.float64)
           - zp.astype(np.float64)[:, None, None, None]).sum(axis=(1, 2, 3))
    sc = (0.01 * scales.astype(np.float64)).astype(np.float32)
    bi = (bias.astype(np.float64)
          - 0.07 * scales.astype(np.float64) * g64).astype(np.float32)
    return sc, bi


def kernel(**inputs) -> np.ndarray:
    x = np.ascontiguousarray(np.asarray(inputs["inputVec"], dtype=np.int8))
    w = np.asarray(inputs["weight"], dtype=np.int8)
    scales = np.asarray(inputs["scales"], dtype=np.float32)
    zp = np.asarray(inputs["zeropoints"], dtype=np.int32)
    bias = np.asarray(inputs["bias"], dtype=np.float32)
    assert x.shape == (N, CIN, H, W) and w.shape == (COUT, CIN, KH, KW)

    wt = _prep_weights(w, zp)
    sc, bi = _prep_scalars(w, zp, scales, bias)

    if "nc" not in _CACHE:
        _CACHE["nc"] = _build_program()
    nc = _CACHE["nc"]

    in_maps = [
        {"x": x[c * NPER:(c + 1) * NPER], "wt": wt, "scales": sc, "bias": bi}
        for c in range(NCORES)
    ]
    res = run_bass_kernel_spmd(nc, in_maps, list(range(NCORES)))
    out = np.concatenate([res.results[c]["out"] for c in range(NCORES)], axis=0)
    return out
